# revision 25
# baseline (speedup 1.0000x reference)
"""Trainium2 Bass kernel for the 4-layer tiny CNN (conv5x5+BN+ReLU+AvgPool+Hardtanh x3, conv4x4+BN1d).

Strategy: pure data parallel over batch (1024 -> 128 images per core on 8 cores).

L1 runs in full 128x128 PE mode (K=120 packs 2 dx-shifts x 5 dy x 4 imgs x 3 ci),
as 32 groups of 4 images; host pre-shifts the input into the x60 layout.

L2/L3/L4 run in 64x64 PE-tiling mode: 4 concurrent 64x64 tiles (2 row groups x
2 col groups) double effective utilization for the 32-channel convs. Row groups
hold duplicated activation copies (dup via SBUF->SBUF DMA); the two row-group
partial sums land in separate PSUM banks and are merged during evacuation with
a fused (b0 + bias) + b1 DVE op. Per round, two conv taps (one per row group)
are applied to N=512 streams; 25 taps take 13 rounds instead of 25 matmuls.

BN is folded into conv weights host-side; the 0.25 avg-pool scale is folded
into W1/W2/W3. ReLU rides the scalar engine (or DVE tensor_scalar for one L1
half); 2x2 avg-pool is two strided vector adds; hardtanh clip is a
tensor_scalar_min writing fp16 activations.
"""
import sys
sys.path.insert(0, '/opt/trn_rl_repo')
import numpy as np

EPS = 1e-5
NCORES = 8
BPC = 128          # images per core
NG = 32            # L1 groups of 4 images per core
ND = 16            # L2 double-sets (8 images = 2 groups)
NSB = 4            # superblocks of 32 images
F1 = 1296          # L1 padded free size (36-wide rows)
OFF1 = 72          # L1 front margin
S2 = 400           # L2 slot: 20x20 padded image
FX2 = 1600         # x2 free size: 4 slots of 400
S3 = 144           # L3 slot: 12x12 padded image
FX3 = 2304         # x3 free size: 16 slots of 144

_NC = None


def _build():
    import concourse.bass as bass
    import concourse.mybir as mybir
    import concourse.tile as tile
    from concourse import bacc

    f32 = mybir.dt.float32
    f16 = mybir.dt.float16
    Relu = mybir.ActivationFunctionType.Relu
    Identity = mybir.ActivationFunctionType.Identity
    ADD = mybir.AluOpType.add
    MAX = mybir.AluOpType.max
    AP = bass.AP

    nc = bacc.Bacc("TRN2", target_bir_lowering=False, debug=False)
    x60d = nc.declare_dram_parameter("x60", [128, NG * F1], f16, isOutput=False)
    w1d = nc.declare_dram_parameter("w1bd", [128, 3 * 128], f16, isOutput=False)
    w2d = nc.declare_dram_parameter("w2til", [128, 13 * 64], f16, isOutput=False)
    w3d = nc.declare_dram_parameter("w3til", [128, 26 * 64], f16, isOutput=False)
    w4d = nc.declare_dram_parameter("w4til", [128, 16 * 64], f16, isOutput=False)
    balld = nc.declare_dram_parameter("ball", [128, 4], f32, isOutput=False)
    outd = nc.declare_dram_parameter("out", [128, 32], f32, isOutput=True)

    with tile.TileContext(nc) as tc:
        with tc.tile_pool(name="consts", bufs=1) as cpool, \
             tc.tile_pool(name="persist", bufs=1) as qpool, \
             tc.tile_pool(name="work", bufs=2) as wpool, \
             tc.tile_pool(name="xin", bufs=32) as xpool, \
             tc.tile_pool(name="ps", bufs=8, space="PSUM") as psp:

            # ---- constants ----
            w1sb = cpool.tile([128, 3 * 128], f16, name="w1sb")
            w2sb = cpool.tile([128, 13 * 64], f16, name="w2sb")
            w3sb = cpool.tile([128, 26 * 64], f16, name="w3sb")
            w4sb = cpool.tile([128, 16 * 64], f16, name="w4sb")
            ball = cpool.tile([128, 4], f32, name="ball")
            nc.sync.dma_start(out=w1sb[:], in_=w1d.ap())
            nc.sync.dma_start(out=ball[:], in_=balld.ap())
            nc.gpsimd.dma_start(out=w2sb[:], in_=w2d.ap())
            nc.gpsimd.dma_start(out=w3sb[:], in_=w3d.ap())
            nc.gpsimd.dma_start(out=w4sb[:], in_=w4d.ap())

            # ---- persistent activation tiles ----
            x2t = [qpool.tile([128, FX2], f16, name=f"x2_{d}") for d in range(ND)]
            x3t = [qpool.tile([128, FX3], f16, name=f"x3_{s}") for s in range(NSB)]
            x4t = qpool.tile([128, 1024], f16, name="x4")
            osb = qpool.tile([128, 32], f32, name="osb")
            # only natural-write slots need zero margins; dup DMAs copy full
            # slots (margins included) into the other positions.
            for t in x2t:
                nc.gpsimd.memset(AP(t.tensor, 0, [[FX2, 128], [800, 2], [1, S2]]),
                                 0.0)
            for t in x3t:
                nc.gpsimd.memset(AP(t.tensor, 0, [[FX3, 64], [288, 8], [1, S3]]),
                                 0.0)
                nc.gpsimd.memset(AP(t.tensor, 64 * FX3 + S3,
                                    [[FX3, 64], [288, 8], [1, S3]]), 0.0)

            # ---- PE warm-up (128-mode, values irrelevant) ----
            psw = psp.tile([128, 512], f32, tag="ps", name="psw")
            for _ in range(8):
                nc.tensor.matmul(psw[:, 0:384], w1sb[:, 0:128], w1sb[:],
                                 start=True, stop=True)

            # ---- preload x60: per-group descriptors on 2 queues (measured
            # fastest config: ~258GB/s aggregate) ----
            x60s = []
            for g in range(NG):
                x60 = xpool.tile([128, F1], f16, tag="x60", name=f"x60_{g}")
                eng = nc.sync if g % 2 == 0 else nc.scalar
                eng.dma_start(out=x60[:],
                              in_=AP(x60d.ap().tensor, g * F1,
                                     [[NG * F1, 128], [1, F1]]))
                x60s.append(x60)

            # ================= L1: conv5x5 3->32 + pool, 128-mode =============
            def emit_l1(g):
                ch = x60s[g]
                base = 0
                ps1a = psp.tile([128, 512], f32, tag="ps", name="ps1a")
                ps1b = psp.tile([128, 512], f32, tag="ps", name="ps1b")
                for hy, ps in ((0, ps1a), (1, ps1b)):
                    for d2 in range(3):
                        rhs = AP(ch.tensor, base + OFF1 + 2 * d2 + hy * 16 * 36,
                                 [[F1, 128], [36, 16], [1, 32]])
                        nc.tensor.matmul(ps[:], w1sb[:, d2 * 128:(d2 + 1) * 128],
                                         rhs, start=(d2 == 0), stop=(d2 == 2))

                r1 = wpool.tile([128, 1024], f16, tag="r1", name="r1")
                nc.scalar.activation(r1[:, 0:512], ps1a[:], Relu,
                                     bias=ball[:, 0:1], scale=1.0)
                nc.scalar.activation(r1[:, 512:1024], ps1b[:], Relu,
                                     bias=ball[:, 0:1], scale=1.0)

                t1 = wpool.tile([128, 512], f16, tag="t1", name="t1")
                r1v = r1.rearrange("p (a b) -> p a b", a=32)
                nc.vector.tensor_add(t1.rearrange("p (a b) -> p a b", a=32),
                                     r1v[:, :, 0::2], r1v[:, :, 1::2])
                t2 = wpool.tile([128, 256], f16, tag="t2", name="t2")
                t1v = t1.rearrange("p (a b) -> p a b", a=32)
                nc.vector.tensor_add(t2.rearrange("p (a b) -> p a b", a=16),
                                     t1v[:, 0::2, :], t1v[:, 1::2, :])

                # copy0 layout [A-p0, A-p1, B-p0, B-p1]; copy1 [A-p1, A-p0, ...]
                # -> pair0 (lanes 0-63) and pair1 (lanes 64-127) write the SAME
                # free offset: one min op covers both.
                d, h = divmod(g, 2)
                t2v = t2.rearrange("p (a b) -> p a b", a=16)
                nc.vector.tensor_scalar_min(
                    AP(x2t[d].tensor, h * 800 + 42,
                       [[FX2, 128], [20, 16], [1, 16]]),
                    t2v[:], 1.0)
                if h == 1:
                    # batched dups: fill each copy's missing slots (pos 400,1200)
                    nc.gpsimd.dma_start(
                        out=AP(x2t[d].tensor, S2, [[FX2, 64], [800, 2], [1, S2]]),
                        in_=AP(x2t[d].tensor, 64 * FX2, [[FX2, 64], [800, 2], [1, S2]]))
                    nc.gpsimd.dma_start(
                        out=AP(x2t[d].tensor, 64 * FX2 + S2,
                               [[FX2, 64], [800, 2], [1, S2]]),
                        in_=AP(x2t[d].tensor, 0, [[FX2, 64], [800, 2], [1, S2]]))

            # ================= L2: conv5x5 32->32 + pool, 64x64 tiles =========
            # Two-stage evac pipeline: stage A (psum merge) runs right after the
            # dset's matmuls; stage B (relu+pools+clip+dups) is deferred until
            # the next dset's matmuls are emitted, so no engine queue blocks at
            # its head waiting on a cross-engine dependency.
            def emit_l2_mm(d):
                pa = psp.tile([128, 512], f32, tag="ps", name="pl2a")
                pb = psp.tile([128, 512], f32, tag="ps", name="pl2b")
                banks = (pa, pb)
                for r in range(13):
                    for I in range(2):
                        t = 2 * r + I
                        if t > 24:
                            continue
                        lhs = w2sb[64 * I:64 * I + 64, r * 64:(r + 1) * 64]
                        stop = (r == 12) if I == 0 else (r == 11)
                        for J in range(2):
                            slot = J if I == 0 else 1 - J
                            rhs = AP(x2t[d].tensor,
                                     64 * I * FX2 + slot * S2
                                     + (t // 5) * 20 + (t % 5),
                                     [[FX2, 64], [800, 2], [20, 16], [1, 16]])
                            nc.tensor.matmul(banks[I][64 * J:64 * J + 64, :],
                                             lhs, rhs, start=(r == 0), stop=stop,
                                             tile_position=(64 * I, 64 * J))
                return pa, pb

            def emit_l2_merge(d, pa, pb):
                tmpb = wpool.tile([128, 512], f16, tag="mb2", name="mb2")
                nc.scalar.activation(tmpb[:], pb[:], Identity,
                                     bias=ball[:, 1:2], scale=1.0)
                tmp = wpool.tile([128, 512], f16, tag="m2", name="m2")
                nc.vector.scalar_tensor_tensor(tmp[:], pa[:], 0.0, tmpb[:],
                                               ADD, ADD)
                return tmp

            def emit_l2_tail(d, tmp):
                sb, k = divmod(d, 4)
                r2 = wpool.tile([128, 512], f16, tag="r2", name="r2")
                nc.scalar.activation(r2[:], tmp[:], Relu, scale=1.0)
                t3 = wpool.tile([128, 256], f16, tag="t3", name="t3")
                r2v = r2.rearrange("p (s a b) -> p s a b", s=2, a=16)
                nc.gpsimd.tensor_add(
                    t3.rearrange("p (s a b) -> p s a b", s=2, a=16),
                    r2v[:, :, :, 0::2], r2v[:, :, :, 1::2])
                t4 = wpool.tile([128, 128], f16, tag="t4", name="t4")
                t3v = t3.rearrange("p (s a b) -> p s a b", s=2, a=16)
                nc.gpsimd.tensor_add(
                    t4.rearrange("p (s a b) -> p s a b", s=2, a=8),
                    t3v[:, :, 0::2, :], t3v[:, :, 1::2, :])

                t4v = t4.rearrange("p (s a b) -> p s a b", s=2, a=8)
                nc.vector.tensor_scalar_min(
                    AP(x3t[sb].tensor, (4 * k) * S3 + 26,
                       [[FX3, 64], [2 * S3, 2], [12, 8], [1, 8]]),
                    t4v[0:64], 1.0)
                nc.vector.tensor_scalar_min(
                    AP(x3t[sb].tensor, 64 * FX3 + (4 * k + 1) * S3 + 26,
                       [[FX3, 64], [2 * S3, 2], [12, 8], [1, 8]]),
                    t4v[64:128], 1.0)
                if k == 3:
                    # batched dups: copy1 <- copy0 even slots, copy0 <- odd
                    nc.gpsimd.dma_start(
                        out=AP(x3t[sb].tensor, 64 * FX3,
                               [[FX3, 64], [288, 8], [1, S3]]),
                        in_=AP(x3t[sb].tensor, 0, [[FX3, 64], [288, 8], [1, S3]]))
                    nc.gpsimd.dma_start(
                        out=AP(x3t[sb].tensor, S3, [[FX3, 64], [288, 8], [1, S3]]),
                        in_=AP(x3t[sb].tensor, 64 * FX3 + S3,
                               [[FX3, 64], [288, 8], [1, S3]]))

            # ================= L3: conv5x5 32->64 + pool, 64x64 tiles =========
            def emit_l3(sb):
                p3 = [[psp.tile([128, 512], f32, tag="ps", name=f"pl3_{I}{c}")
                       for c in range(2)] for I in range(2)]
                for r in range(13):
                    for I in range(2):
                        t = 2 * r + I
                        if t > 24:
                            continue
                        stop = (r == 12) if I == 0 else (r == 11)
                        for J in range(2):
                            lhs = w3sb[64 * I:64 * I + 64,
                                       (2 * r + J) * 64:(2 * r + J) * 64 + 64]
                            for c in range(2):
                                rhs = AP(x3t[sb].tensor,
                                         64 * I * FX3 + c * 8 * S3
                                         + (t // 5) * 12 + (t % 5),
                                         [[FX3, 64], [S3, 8], [12, 8], [1, 8]])
                                nc.tensor.matmul(
                                    p3[I][c][64 * J:64 * J + 64, :],
                                    lhs, rhs, start=(r == 0), stop=stop,
                                    tile_position=(64 * I, 64 * J))

                # hand-pipelined evac: merges for both c first, then tails
                tmps = []
                for c in range(2):
                    tmpb = wpool.tile([128, 512], f16, tag="mb3", name="mb3")
                    nc.scalar.activation(tmpb[:], p3[1][c][:], Identity,
                                         bias=ball[:, 2:3], scale=1.0)
                    tmp = wpool.tile([128, 512], f16, tag="m3", name="m3")
                    nc.vector.scalar_tensor_tensor(tmp[:], p3[0][c][:], 0.0,
                                                   tmpb[:], ADD, ADD)
                    tmps.append(tmp)
                r3s = []
                for c in range(2):
                    r3 = wpool.tile([128, 512], f16, tag="r3", name="r3")
                    nc.scalar.activation(r3[:], tmps[c][:], Relu, scale=1.0)
                    r3s.append(r3)
                t6s = []
                for c in range(2):
                    t5 = wpool.tile([128, 256], f16, tag="t5", name="t5")
                    r3v = r3s[c].rearrange("p (s a b) -> p s a b", s=8, a=8)
                    nc.gpsimd.tensor_add(
                        t5.rearrange("p (s a b) -> p s a b", s=8, a=8),
                        r3v[:, :, :, 0::2], r3v[:, :, :, 1::2])
                    t6 = wpool.tile([128, 128], f16, tag="t6", name="t6")
                    t5v = t5.rearrange("p (s a b) -> p s a b", s=8, a=8)
                    nc.gpsimd.tensor_add(
                        t6.rearrange("p (s a b) -> p s a b", s=8, a=4),
                        t5v[:, :, 0::2, :], t5v[:, :, 1::2, :])
                    t6s.append(t6)
                for c in range(2):
                    t6v = t6s[c].rearrange("p (s a b) -> p s a b", s=8, a=4)
                    nc.vector.tensor_scalar_min(
                        AP(x4t.tensor, sb * 256 + c * 128,
                           [[1024, 128], [16, 8], [4, 4], [1, 4]]),
                        t6v[:], 1.0)

            # ---- emission order: L1 all; L2 two-stage pipelined; L3 slid -----
            for g in range(NG):
                emit_l1(g)
            # quad order Q3,Q0,Q1,Q2 so the final superblock's inputs are
            # ready long before its matmuls; sbs interleave as 3,0,1,2.
            dlist = [12, 13, 14, 15, 0, 1, 2, 3, 4, 5, 6, 7, 8, 9, 10, 11]
            pend = None
            for pos, d in enumerate(dlist):
                pa, pb = emit_l2_mm(d)
                if pend is not None:
                    emit_l2_tail(pend[0], pend[1])
                tmp = emit_l2_merge(d, pa, pb)
                pend = (d, tmp)
                if pos == 7:
                    emit_l3(3)
                elif pos == 11:
                    emit_l3(0)
                elif pos == 15:
                    emit_l3(1)
            emit_l2_tail(pend[0], pend[1])
            emit_l3(2)

            # ================= L4: fc conv4x4 + BN1d, 64x64 tiles =============
            p4a = psp.tile([128, 512], f32, tag="ps", name="pl4a")
            p4b = psp.tile([128, 512], f32, tag="ps", name="pl4b")
            p4 = (p4a, p4b)
            for tap in range(16):
                for I in range(2):
                    lhs = w4sb[64 * I:64 * I + 64, tap * 64:(tap + 1) * 64]
                    for J in range(2):
                        rhs = AP(x4t.tensor, 64 * I * 1024 + 32 * J * 16 + tap,
                                 [[1024, 64], [16, 32]])
                        nc.tensor.matmul(p4[I][64 * J:64 * J + 64, 0:32],
                                         lhs, rhs,
                                         start=(tap == 0), stop=(tap == 15),
                                         tile_position=(64 * I, 64 * J))
            ob = qpool.tile([128, 32], f32, name="ob")
            nc.scalar.activation(ob[:], p4b[:, 0:32], Identity,
                                 bias=ball[:, 3:4], scale=1.0)
            nc.vector.scalar_tensor_tensor(osb[:], p4a[:, 0:32], 0.0, ob[:],
                                           ADD, ADD)
            nc.sync.dma_start(out=outd.ap(), in_=osb[:])

    nc.compile()
    return nc


def _fold(g, b, m, v):
    s = (g / np.sqrt(v + EPS)).astype(np.float32)
    return s, (b - m * s).astype(np.float32)


def _prep_consts(w1, g1, b1, m1, v1, w2, g2, b2, m2, v2,
                 w3, g3, b3, m3, v3, w4, g4, b4, m4, v4):
    s1, t1 = _fold(g1, b1, m1, v1)
    s2, t2 = _fold(g2, b2, m2, v2)
    s3, t3 = _fold(g3, b3, m3, v3)
    s4, t4 = _fold(g4, b4, m4, v4)
    w1f = (0.25 * w1 * s1[:, None, None, None]).astype(np.float32)  # [32,3,5,5]
    w2f = (0.25 * w2 * s2[:, None, None, None]).astype(np.float32)  # [32,32,5,5]
    w3f = (0.25 * w3 * s3[:, None, None, None]).astype(np.float32)  # [64,32,5,5]
    w4f = (w4 * s4[:, None, None, None]).astype(np.float32)         # [10,64,4,4]

    # L1: [ (dxl,dy,i,ci)=120, d2*128 + (i2*32+co) ], tap dx = 2*d2 + dxl
    w1bd = np.zeros((2, 5, 4, 3, 3, 4, 32), np.float32)
    wt1 = w1f.transpose(3, 2, 1, 0)  # dx,dy,ci,co
    for i in range(4):
        for d2 in range(3):
            for dxl in range(2):
                dx = 2 * d2 + dxl
                if dx < 5:
                    w1bd[dxl, :, i, :, d2, i, :] = wt1[dx]
    w1bd = w1bd.reshape(120, 3 * 128)
    w1bd = np.concatenate([w1bd, np.zeros((8, 3 * 128), np.float32)], axis=0)
    w1bd = w1bd.astype(np.float16)

    # L2 tiled: rows 64I+(i2*32+ci), cols r*64+(i2*32+co), tap t=2r+I
    w2til = np.zeros((2, 2, 32, 13, 2, 32), np.float32)  # I,i2,ci,r,i2',co
    wt2 = w2f.transpose(1, 0, 2, 3).reshape(32, 32, 25)  # ci,co,t
    for I in range(2):
        for r in range(13):
            t = 2 * r + I
            if t > 24:
                continue
            for i2 in range(2):
                w2til[I, i2, :, r, i2, :] = wt2[:, :, t]
    w2til = w2til.reshape(128, 13 * 64).astype(np.float16)

    # L3 tiled: rows 64I+(i2*32+ci), cols (2r+J)*64+(i2*32+coh), tap t=2r+I
    w3til = np.zeros((2, 2, 32, 13, 2, 2, 32), np.float32)  # I,i2,ci,r,J,i2',coh
    wt3 = w3f.transpose(1, 0, 2, 3).reshape(32, 2, 32, 25)  # ci,J,coh,t
    for I in range(2):
        for r in range(13):
            t = 2 * r + I
            if t > 24:
                continue
            for i2 in range(2):
                for J in range(2):
                    w3til[I, i2, :, r, J, i2, :] = wt3[:, J, :, t]
    w3til = w3til.reshape(128, 26 * 64).astype(np.float16)

    # L4 tiled: rows 64I+(i2*32+c'), cols tap*64+(i2*32+co); c = 32I+c'
    w4til = np.zeros((2, 2, 32, 16, 2, 32), np.float32)  # I,i2,c',tap,i2',co
    wt4 = w4f.reshape(10, 2, 32, 16)  # co, I(c-half), c', tap
    for I in range(2):
        for i2 in range(2):
            w4til[I, i2, :, :, i2, 0:10] = wt4[:, I, :, :].transpose(1, 2, 0)
    w4til = w4til.reshape(128, 16 * 64).astype(np.float16)

    ball = np.zeros((128, 4), np.float32)
    p = np.arange(128)
    ball[:, 0] = 0.25 * np.tile(t1, 4)
    ball[:, 1] = 0.25 * t2[p % 32]
    ball[:, 2] = 0.25 * t3[32 * (p // 64) + (p % 32)]
    ball[:, 3] = np.where(p % 32 < 10, t4[np.minimum(p % 32, 9)], 0.0)
    return dict(w1bd=w1bd, w2til=w2til, w3til=w3til, w4til=w4til, ball=ball)


def _prep_x60(xc):
    # xc: [128, 3, 32, 32] -> [32, 120, F1]; partition (dxl,dy,i,ci), 36-wide rows
    xp = np.zeros((NG, 4, 3, 40, 36), np.float32)
    xp[:, :, :, 2:34, 2:34] = xc.reshape(NG, 4, 3, 32, 32)
    xf = xp.reshape(NG, 4, 3, 40 * 36)
    out = np.zeros((NG, 128, F1), np.float32)
    v = out[:, 0:120, :].reshape(NG, 2, 5, 4, 3, F1)
    for dxl in range(2):
        for dy in range(5):
            off = 36 * dy + dxl
            v[:, dxl, dy, :, :, OFF1:OFF1 + 1223] = xf[:, :, :, off:off + 1223]
    # partition-major: [128, NG*F1] so one DMA chunk covers 8 groups
    return np.ascontiguousarray(
        out.transpose(1, 0, 2).reshape(128, NG * F1)).astype(np.float16)


def kernel(**inputs):
    global _NC
    from concourse.bass_utils import run_bass_kernel_spmd

    x = np.ascontiguousarray(np.asarray(inputs["x"], dtype=np.float32))
    consts = _prep_consts(
        inputs["w1"], inputs["g1"], inputs["b1"], inputs["m1"], inputs["v1"],
        inputs["w2"], inputs["g2"], inputs["b2"], inputs["m2"], inputs["v2"],
        inputs["w3"], inputs["g3"], inputs["b3"], inputs["m3"], inputs["v3"],
        inputs["w4"], inputs["g4"], inputs["b4"], inputs["m4"], inputs["v4"])
    consts = {k: np.ascontiguousarray(v) for k, v in consts.items()}

    if _NC is None:
        _NC = _build()

    in_maps = []
    for c in range(NCORES):
        m = dict(consts)
        m["x60"] = _prep_x60(x[c * BPC:(c + 1) * BPC])
        in_maps.append(m)

    res = run_bass_kernel_spmd(_NC, in_maps, list(range(NCORES)))
    outs = []
    for c in range(NCORES):
        o = res.results[c]["out"].reshape(2, 2, 32, 32)  # [J, i2, co, n]
        o = o.transpose(0, 3, 1, 2).reshape(128, 32)     # img = 64J + 2n + i2
        outs.append(o[:, 0:10])
    return np.concatenate(outs, axis=0).astype(np.float32)
